# revision 1
# baseline (speedup 1.0000x reference)
"""MeshConv (GNN message passing) Bass kernel for 8 trn2 NeuronCores.

Strategy
--------
Shard (batch, edge-half): core c handles batch c//2, edge half c%2.
Host marshals per-batch token-major bf16 tables x[b].T (E, 32) and
per-core index lists. Device gathers 5 neighbor tokens per edge with
SWDGE indirect DMA ([128,1]-offset form: one token per partition per
call), combines them on DVE (sums + abs-diffs of the one-ring), PE
transposes token-major G tiles to feature-major and contracts with the
stacked conv weights (K=128 block: [f0, f1+f3, f2+f4, |f1-f3|] and a
K=32 block: |f2-f4|), accumulating in PSUM.  ACT adds bias and the
result is DMAed out as (64, E_half) f32 per core.
"""

import sys

sys.path.insert(0, "/opt/trn_rl_repo")

import numpy as np
import ml_dtypes

B, C_IN, C_OUT, E, KK = 4, 32, 64, 200000, 5
NCORES = 8
EH = E // 2  # edges per core


def _plan(eh, gpc):
    ngroups = -(-eh // 128)
    nchunk = -(-ngroups // gpc)
    edges_pad = nchunk * gpc * 128
    return nchunk, edges_pad


_PROG_CACHE = {}


def _build(table_rows, nchunk, gpc, reps=1, passthrough=False):
    key = (table_rows, nchunk, gpc, reps, passthrough)
    if key in _PROG_CACHE:
        return _PROG_CACHE[key]
    import concourse.bass as bass
    import concourse.bacc as bacc
    import concourse.tile as tile
    from concourse import mybir
    from concourse.masks import make_identity

    dt = mybir.dt
    nc = bacc.Bacc("TRN2", target_bir_lowering=False, debug=False)
    table = nc.dram_tensor("table", [table_rows, C_IN], dt.bfloat16, kind="ExternalInput")
    idx_d = nc.dram_tensor("idx", [nchunk, 128, gpc * 5], dt.int32, kind="ExternalInput")
    wmain_d = nc.dram_tensor("wmain", [128, C_OUT], dt.bfloat16, kind="ExternalInput")
    wd2_d = nc.dram_tensor("wd2", [32, C_OUT], dt.bfloat16, kind="ExternalInput")
    bias_d = nc.dram_tensor("bias", [C_OUT, 1], dt.float32, kind="ExternalInput")
    out_d = nc.dram_tensor("out", [C_OUT, nchunk * gpc * 128], dt.float32, kind="ExternalOutput")

    AT = mybir.AluOpType
    if passthrough:
        # trivial program with identical external I/O — used by test.py to
        # measure the axon dispatch + transfer floor
        with tile.TileContext(nc) as tc:
            with tc.tile_pool(name="pt", bufs=1) as ptp:
                z = ptp.tile([C_OUT, gpc * 128], dt.float32)
                nc.vector.memset(z[:], 0.0)
                for ch in range(nchunk):
                    nc.sync.dma_start(
                        out_d[:, ch * gpc * 128 : (ch + 1) * gpc * 128], z[:]
                    )
        nc.compile()
        _PROG_CACHE[key] = nc
        return nc
    with tile.TileContext(nc) as tc:
        with (
            tc.tile_pool(name="const", bufs=1) as cp,
            tc.tile_pool(name="idxp", bufs=4) as ip,
            tc.tile_pool(name="tsp", bufs=3) as tsp,
            tc.tile_pool(name="ftp", bufs=3) as ftp,
            tc.tile_pool(name="scp", bufs=3) as scp,
            tc.tile_pool(name="rhsp", bufs=6) as rp,
            tc.tile_pool(name="obp", bufs=3) as obp,
            tc.tile_pool(name="psp", bufs=3, space="PSUM") as pp,
            tc.tile_pool(name="psp2", bufs=2, space="PSUM") as pp2,
        ):
            wmain = cp.tile([128, C_OUT], dt.bfloat16)
            nc.sync.dma_start(wmain[:], wmain_d[:])
            wd2 = cp.tile([32, C_OUT], dt.bfloat16)
            nc.sync.dma_start(wd2[:], wd2_d[:])
            bias = cp.tile([C_OUT, 1], dt.float32)
            nc.sync.dma_start(bias[:], bias_d[:])
            ident = cp.tile([128, 128], dt.bfloat16)
            make_identity(nc, ident[:])

            for _ in range(reps):
                for ch in range(nchunk):
                    it = ip.tile([128, gpc * 5], dt.int32, tag="it")
                    nc.sync.dma_start(it[:], idx_d[ch])
                    ts = tsp.tile([128, gpc, 160], dt.bfloat16, tag="ts")
                    ft = ftp.tile([128, gpc, 128], dt.bfloat16, tag="ft")
                    for g in range(gpc):
                        nc.gpsimd.indirect_dma_start(
                            out=ts[:, g, 0:32],
                            out_offset=None,
                            in_=table[:],
                            in_offset=bass.IndirectOffsetOnAxis(ap=it[:, g * 5 : g * 5 + 1], axis=0),
                        )
                        for j in range(1, 5):
                            nc.gpsimd.indirect_dma_start(
                                out=ft[:, g, (j - 1) * 32 : j * 32],
                                out_offset=None,
                                in_=table[:],
                                in_offset=bass.IndirectOffsetOnAxis(
                                    ap=it[:, g * 5 + j : g * 5 + j + 1], axis=0
                                ),
                            )
                    f1 = ft[:, :, 0:32]
                    f2 = ft[:, :, 32:64]
                    f3 = ft[:, :, 64:96]
                    f4 = ft[:, :, 96:128]
                    sc1 = scp.tile([128, gpc, 32], dt.bfloat16, tag="sc1")
                    sc2 = scp.tile([128, gpc, 32], dt.bfloat16, tag="sc2")
                    nc.vector.tensor_tensor(out=ts[:, :, 32:64], in0=f1, in1=f3, op=AT.add)
                    nc.vector.tensor_tensor(out=ts[:, :, 64:96], in0=f2, in1=f4, op=AT.add)
                    nc.vector.tensor_tensor(out=sc1[:], in0=f1, in1=f3, op=AT.subtract)
                    nc.scalar.activation(
                        ts[:, :, 96:128], sc1[:], mybir.ActivationFunctionType.Abs
                    )
                    nc.vector.tensor_tensor(out=sc2[:], in0=f2, in1=f4, op=AT.subtract)
                    nc.scalar.activation(
                        ts[:, :, 128:160], sc2[:], mybir.ActivationFunctionType.Abs
                    )
                    ob = obp.tile([C_OUT, gpc * 128], dt.float32, tag="ob")
                    quad = 4 if gpc % 4 == 0 else 1
                    for q in range(gpc // quad):
                        gs = range(q * quad, (q + 1) * quad)
                        t1 = pp.tile([128, quad * 128], dt.bfloat16, tag="t1")
                        t2 = pp2.tile([32, quad * 128], dt.bfloat16, tag="t2")
                        for i, g in enumerate(gs):
                            nc.tensor.transpose(
                                t1[:, i * 128 : (i + 1) * 128], ts[:, g, 0:128], ident[:]
                            )
                            nc.tensor.transpose(
                                t2[:, i * 128 : (i + 1) * 128], ts[:, g, 128:160], ident[:]
                            )
                        r1 = rp.tile([128, quad * 128], dt.bfloat16, tag="r1")
                        nc.scalar.activation(
                            r1[:], t1[:], mybir.ActivationFunctionType.Copy
                        )
                        r2 = rp.tile([32, quad * 128], dt.bfloat16, tag="r2")
                        nc.vector.tensor_copy(r2[:], t2[:])
                        o = pp.tile([C_OUT, quad * 128], dt.float32, tag="o")
                        nc.tensor.matmul(o[:], wmain[:], r1[:], start=True, stop=False)
                        nc.tensor.matmul(o[:], wd2[:], r2[:], start=False, stop=True)
                        nc.vector.tensor_scalar(
                            out=ob[:, q * quad * 128 : (q + 1) * quad * 128],
                            in0=o[:],
                            scalar1=bias[:],
                            scalar2=None,
                            op0=AT.add,
                        )
                    nc.sync.dma_start(out_d[:, ch * gpc * 128 : (ch + 1) * gpc * 128], ob[:])
    nc.compile()
    _PROG_CACHE[key] = nc
    return nc


def _marshal_core(x_b, gi_core, eh, nchunk, gpc):
    """Per-core inputs: token-major bf16 table + chunked index tile."""
    edges_pad = nchunk * gpc * 128
    gi_pad = np.zeros((edges_pad, 5), np.int32)
    gi_pad[:eh] = gi_core
    idx = (
        gi_pad.reshape(nchunk, gpc, 128, 5)
        .transpose(0, 2, 1, 3)
        .reshape(nchunk, 128, gpc * 5)
    )
    table = np.ascontiguousarray(x_b.T).astype(ml_dtypes.bfloat16)
    return table, np.ascontiguousarray(idx)


def _marshal_weights(W, b):
    Wk = np.asarray(W)[:, :, 0, :]  # (C_OUT, C_IN, 5)
    wmain = np.zeros((128, C_OUT), np.float32)
    for k in range(4):
        wmain[32 * k : 32 * (k + 1), :] = Wk[:, :, k].T
    wd2 = np.ascontiguousarray(Wk[:, :, 4].T)
    bias = np.asarray(b).reshape(C_OUT, 1).astype(np.float32)
    return (
        wmain.astype(ml_dtypes.bfloat16),
        wd2.astype(ml_dtypes.bfloat16),
        bias,
    )


def _run(x, Gi, W, b, gpc=16, reps=1, passthrough=False):
    from concourse.bass_utils import run_bass_kernel_spmd

    x = np.asarray(x)
    Gi = np.asarray(Gi)
    nchunk, _ = _plan(EH, gpc)
    nc = _build(E, nchunk, gpc, reps, passthrough)
    wmain, wd2, bias = _marshal_weights(W, b)
    tables = {}
    in_maps = []
    for c in range(NCORES):
        bb, h = divmod(c, 2)
        if bb not in tables:
            tables[bb] = np.ascontiguousarray(x[bb].T).astype(ml_dtypes.bfloat16)
        gi_core = Gi[bb, h * EH : (h + 1) * EH]
        edges_pad = nchunk * gpc * 128
        gi_pad = np.zeros((edges_pad, 5), np.int32)
        gi_pad[:EH] = gi_core
        idx = (
            gi_pad.reshape(nchunk, gpc, 128, 5)
            .transpose(0, 2, 1, 3)
            .reshape(nchunk, 128, gpc * 5)
        )
        in_maps.append(
            {
                "table": tables[bb],
                "idx": np.ascontiguousarray(idx),
                "wmain": wmain,
                "wd2": wd2,
                "bias": bias,
            }
        )
    res = run_bass_kernel_spmd(nc, in_maps, core_ids=list(range(NCORES)))
    out = np.empty((B, C_OUT, E, 1), np.float32)
    for c in range(NCORES):
        bb, h = divmod(c, 2)
        out[bb, :, h * EH : (h + 1) * EH, 0] = res.results[c]["out"][:, :EH]
    return out


def kernel(x, Gi, W, b):
    return _run(x, Gi, W, b)



# revision 6
# speedup vs baseline: 1.1670x; 1.1670x over previous
"""MeshConv (GNN message passing) Bass kernel for 8 trn2 NeuronCores.

Strategy (v2: two-stage dma_gather)
-----------------------------------
Shard (batch, edge-half): core c handles batch c//2, edge half c%2
(100000 edges each).  Per batch the host builds a 256 B/row token table
(row = 32 bf16 features + zero pad) with a zero row 0, so idx = Gi+1.

Per 2048-edge chunk (10240 gather slots, j-major):
  g1  Host pre-buckets the chunk's slots by 32768-row index window
      (int16 limit).  One InstDMAGatherAnt per window gathers the
      compact list from HBM into a sorted token-major SBUF tile FS
      (one 256 B stripe per slot).  Lists are sentinel-padded to fixed
      per-window caps so one traced program serves every chunk/core.
  g2  One SBUF-source transpose dma_gather un-permutes FS back to
      logical slot order AND transposes to feature-major in one step:
      FL[0:32, q] = features of slot q's token.
  DVE/ACT combine on partitions 0-31: f1+f3, f2+f4, |f1-f3|, |f2-f4|.
  Five accumulating K=32 matmuls (one per conv tap) into PSUM, bias
  added on DVE during PSUM->SBUF copy, result DMAed out as f32.
"""

import sys

sys.path.insert(0, "/opt/trn_rl_repo")

import numpy as np
import ml_dtypes

B, C_IN, C_OUT, E, KK = 4, 32, 64, 200000, 5
NCORES = 8

# --- tunables / derived sizes (module-level so a sim harness can override) ---
CE = 2048  # edges per chunk
WIN = 1 << 15  # index window (int16 limit)


def _sizes():
    EH = E // 2  # edges per core
    R = E + 1  # table rows (row 0 = zeros)
    NW = -(-R // WIN)
    NCH = -(-EH // CE)
    EPAD = NCH * CE
    N2 = 5 * CE
    return EH, R, NW, NCH, EPAD, N2


_PROG_CACHE = {}


def _build(caps, reps=1, passthrough=False):
    key = (tuple(caps), E, CE, reps, passthrough)
    if key in _PROG_CACHE:
        return _PROG_CACHE[key]
    import concourse.bass as bass
    import concourse.bacc as bacc
    import concourse.tile as tile
    from concourse import mybir

    EH, R, NW, NCH, EPAD, N2 = _sizes()
    TS = int(sum(caps))
    TQ = TS // 128
    QN = 512 if CE % 512 == 0 else CE  # matmul column block
    NQ = CE // QN

    dt = mybir.dt
    AT = mybir.AluOpType
    nc = bacc.Bacc("TRN2", target_bir_lowering=False, debug=False)
    table_d = nc.dram_tensor("table", [R, 128], dt.bfloat16, kind="ExternalInput")
    g1_d = nc.dram_tensor("g1idx", [NCH, 128, TS // 16], dt.int16, kind="ExternalInput")
    g2_d = nc.dram_tensor("g2idx", [NCH, 128, N2 // 16], dt.int16, kind="ExternalInput")
    wts_d = nc.dram_tensor("wts", [C_IN, 5 * C_OUT], dt.bfloat16, kind="ExternalInput")
    bias_d = nc.dram_tensor("bias", [C_OUT, 1], dt.float32, kind="ExternalInput")
    out_d = nc.dram_tensor("out", [C_OUT, EPAD], dt.float32, kind="ExternalOutput")

    if passthrough:
        # trivial program with identical external I/O — used by test.py to
        # measure the axon dispatch + transfer floor
        with tile.TileContext(nc) as tc:
            with tc.tile_pool(name="pt", bufs=1) as ptp:
                z = ptp.tile([C_OUT, CE], dt.float32)
                nc.vector.memset(z[:], 0.0)
                for ch in range(NCH):
                    nc.sync.dma_start(out_d[:, ch * CE : (ch + 1) * CE], z[:])
        nc.compile()
        _PROG_CACHE[key] = nc
        return nc

    offs = np.concatenate([[0], np.cumsum(caps)]).astype(int)

    with tile.TileContext(nc) as tc:
        with (
            tc.tile_pool(name="const", bufs=1) as cp,
            tc.tile_pool(name="idxp", bufs=3) as ip,
            tc.tile_pool(name="fsp", bufs=2) as fsp,
            tc.tile_pool(name="flp", bufs=2) as flp,
            tc.tile_pool(name="ttp", bufs=2) as ttp,
            tc.tile_pool(name="obp", bufs=2) as obp,
            tc.tile_pool(name="psp", bufs=4, space="PSUM") as pp,
        ):
            wts = cp.tile([C_IN, 5 * C_OUT], dt.bfloat16)
            nc.sync.dma_start(wts[:], wts_d[:])
            bias = cp.tile([C_OUT, 1], dt.float32)
            nc.sync.dma_start(bias[:], bias_d[:])

            for _ in range(reps):
                for ch in range(NCH):
                    it1 = ip.tile([128, TS // 16], dt.int16, tag="it1")
                    nc.sync.dma_start(it1[:], g1_d[ch])
                    it2 = ip.tile([128, N2 // 16], dt.int16, tag="it2")
                    nc.sync.dma_start(it2[:], g2_d[ch])

                    fs = fsp.tile([128, TQ, 128], dt.bfloat16, tag="fs")
                    for w in range(NW):
                        cw = int(caps[w])
                        lo, hi = w * WIN, min(R, (w + 1) * WIN)
                        nc.gpsimd.dma_gather(
                            fs[:, offs[w] // 128 : offs[w + 1] // 128, :],
                            table_d[lo:hi],
                            it1[:, offs[w] // 16 : offs[w + 1] // 16],
                            cw,
                            cw,
                            128,
                            single_packet=False,
                        )
                    fl = flp.tile([128, 1, N2], dt.bfloat16, tag="fl")
                    nc.gpsimd.dma_gather(
                        fl[:],
                        fs[:],
                        it2[:],
                        N2,
                        N2,
                        128,
                        transpose=True,
                        sbuf_tokens_per_rank=128,
                        sbuf_free_dim_per_rank=256,
                        sbuf_free_dim_pad_per_rank=0,
                        sbuf_byte_offset=0,
                        single_packet=False,
                    )

                    def f(j):
                        return fl[0:C_IN, 0, j * CE : (j + 1) * CE]

                    tt = ttp.tile([C_IN, 4 * CE], dt.bfloat16, tag="tt")
                    sc = ttp.tile([C_IN, 2 * CE], dt.bfloat16, tag="sc")
                    nc.vector.tensor_tensor(
                        out=tt[:, 0:CE], in0=f(1), in1=f(3), op=AT.add
                    )
                    nc.vector.tensor_tensor(
                        out=tt[:, CE : 2 * CE], in0=f(2), in1=f(4), op=AT.add
                    )
                    nc.vector.tensor_tensor(
                        out=sc[:, 0:CE], in0=f(1), in1=f(3), op=AT.subtract
                    )
                    nc.scalar.activation(
                        tt[:, 2 * CE : 3 * CE],
                        sc[:, 0:CE],
                        mybir.ActivationFunctionType.Abs,
                    )
                    nc.vector.tensor_tensor(
                        out=sc[:, CE : 2 * CE], in0=f(2), in1=f(4), op=AT.subtract
                    )
                    nc.scalar.activation(
                        tt[:, 3 * CE : 4 * CE],
                        sc[:, CE : 2 * CE],
                        mybir.ActivationFunctionType.Abs,
                    )

                    ob = obp.tile([C_OUT, CE], dt.float32, tag="ob")
                    for q in range(NQ):
                        ps = pp.tile([C_OUT, QN], dt.float32, tag="ps")
                        nc.tensor.matmul(
                            ps[:],
                            wts[:, 0:C_OUT],
                            fl[0:C_IN, 0, q * QN : (q + 1) * QN],
                            start=True,
                            stop=False,
                        )
                        for k in range(1, 5):
                            nc.tensor.matmul(
                                ps[:],
                                wts[:, k * C_OUT : (k + 1) * C_OUT],
                                tt[:, (k - 1) * CE + q * QN : (k - 1) * CE + (q + 1) * QN],
                                start=False,
                                stop=(k == 4),
                            )
                        nc.vector.tensor_scalar(
                            out=ob[:, q * QN : (q + 1) * QN],
                            in0=ps[:],
                            scalar1=bias[:],
                            scalar2=None,
                            op0=AT.add,
                        )
                    nc.sync.dma_start(out_d[:, ch * CE : (ch + 1) * CE], ob[:])
    nc.compile()
    _PROG_CACHE[key] = nc
    return nc


def _wrap16(a):
    """(NCH, N) int16 -> (NCH, 128, N//16): index i -> [p, i//16] for all
    p % 16 == i % 16 (value wrapped in 16 partitions, replicated 8x)."""
    nch, n = a.shape
    b = a.reshape(nch, n // 16, 16).transpose(0, 2, 1)  # (NCH, 16, N//16)
    return np.ascontiguousarray(np.tile(b, (1, 8, 1)))


def _marshal_core(Gi_core, caps, offs):
    """Build g1 (window-bucketed gather lists) and g2 (un-permute) index
    arrays for one core from its (EH, 5) one-based token indices."""
    EH, R, NW, NCH, EPAD, N2 = _sizes()
    WSH = WIN.bit_length() - 1
    TS = int(sum(caps))
    gp = np.zeros((EPAD, KK), np.int64)
    gp[:EH] = Gi_core
    tok = gp.reshape(NCH, CE, KK).transpose(0, 2, 1).reshape(NCH, N2)
    w = tok >> WSH
    g1 = np.zeros((NCH, TS), np.int16)
    i2 = np.empty((NCH, N2), np.int16)
    for ch in range(NCH):
        tw, tt = w[ch], tok[ch]
        for wi in range(NW):
            m = tw == wi
            nn = int(m.sum())
            g1[ch, offs[wi] : offs[wi] + nn] = (tt[m] - (wi << WSH)).astype(np.int16)
            i2[ch, m] = (offs[wi] + np.arange(nn)).astype(np.int16)
    return _wrap16(g1), _wrap16(i2)


def _counts_core(Gi_core):
    EH, R, NW, NCH, EPAD, N2 = _sizes()
    WSH = WIN.bit_length() - 1
    gp = np.zeros((EPAD, KK), np.int64)
    gp[:EH] = Gi_core
    w = (gp.reshape(NCH, CE, KK).transpose(0, 2, 1).reshape(NCH, N2)) >> WSH
    return np.stack([np.bincount(w[ch], minlength=NW) for ch in range(NCH)])


def _marshal_weights(W, b):
    Wk = np.asarray(W)[:, :, 0, :]  # (C_OUT, C_IN, 5)
    wts = np.zeros((C_IN, 5 * C_OUT), np.float32)
    for k in range(KK):
        wts[:, k * C_OUT : (k + 1) * C_OUT] = Wk[:, :, k].T
    bias = np.asarray(b).reshape(C_OUT, 1).astype(np.float32)
    return wts.astype(ml_dtypes.bfloat16), bias


def _run(x, Gi, W, b, reps=1, passthrough=False):
    from concourse.bass_utils import run_bass_kernel_spmd

    EH, R, NW, NCH, EPAD, N2 = _sizes()
    x = np.asarray(x)
    Gi = np.asarray(Gi)

    gi1 = {}  # one-based token index per core
    for c in range(NCORES):
        bb, h = divmod(c, 2)
        gi1[c] = Gi[bb, h * EH : (h + 1) * EH].astype(np.int64) + 1

    # fixed per-window caps: max bucket size over every (core, chunk)
    cnts = np.stack([_counts_core(gi1[c]) for c in range(NCORES)])  # (8, NCH, NW)
    caps = (-(-cnts.max(axis=(0, 1)) // 128) * 128).astype(int)
    offs = np.concatenate([[0], np.cumsum(caps)]).astype(int)

    nc = _build(caps, reps, passthrough)
    wts, bias = _marshal_weights(W, b)

    tables = {}
    for bb in range(B):
        t = np.zeros((R, 128), ml_dtypes.bfloat16)
        t[1:, 0:C_IN] = x[bb].T.astype(ml_dtypes.bfloat16)
        tables[bb] = t

    in_maps = []
    for c in range(NCORES):
        bb, h = divmod(c, 2)
        g1, g2 = _marshal_core(gi1[c], caps, offs)
        in_maps.append(
            {
                "table": tables[bb],
                "g1idx": g1,
                "g2idx": g2,
                "wts": wts,
                "bias": bias,
            }
        )
    res = run_bass_kernel_spmd(nc, in_maps, core_ids=list(range(NCORES)))
    out = np.empty((B, C_OUT, E, 1), np.float32)
    for c in range(NCORES):
        bb, h = divmod(c, 2)
        out[bb, :, h * EH : (h + 1) * EH, 0] = res.results[c]["out"][:, :EH]
    return out


def kernel(x, Gi, W, b):
    return _run(x, Gi, W, b)


# revision 10
# speedup vs baseline: 6.0341x; 5.1705x over previous
"""MeshConv (GNN message passing) Bass kernel for 8 trn2 NeuronCores.

Strategy (v3: two-stage dma_gather, 4 SWDGE queues, software-pipelined)
-----------------------------------------------------------------------
Shard (batch, edge-half): core c handles batch c//2, edge half c%2
(100000 edges each).  Per batch the host builds a 256 B/row token table
(row = 32 bf16 features + zero pad) with a zero row 0, so idx = Gi+1.

Per 2048-edge chunk (10240 gather slots, j-major):
  g1  Host pre-buckets the chunk's slots by 32768-row index window
      (int16 limit).  One InstDMAGatherAnt per window gathers the
      compact list from HBM into a sorted token-major SBUF tile FS
      (one 256 B stripe per slot).  Lists are sentinel-padded to fixed
      per-window caps so one traced program serves every chunk/core.
  g2  Two SBUF-source transpose dma_gathers (slots j<2 and j>=2)
      un-permute FS back to logical slot order AND transpose to
      feature-major: FL[0:32, q] = features of slot q's token.
  DVE/ACT combine on partitions 0-31: f1+f3, f2+f4, |f1-f3|, |f2-f4|.
  Five accumulating K=32 matmuls (one per conv tap) into PSUM, bias
  added on DVE during PSUM->SBUF copy, result DMAed out as f32.

Gather calls rotate over all 4 SWDGE queues (queue q runs on Q7 core
pair q — parallel descriptor gen + 4 independent descriptor rings).
The rotation (count % 4) is congruent with Tile's 8 DMASW sem lanes so
each lane only ever sees one queue.  Chunks are emitted software-
pipelined (chunk ch's g1 before chunk ch-1's g2+compute) so the Pool
sequencer's wait on FS(ch-1) does not block g1(ch) descriptor gen.
"""

import sys

sys.path.insert(0, "/opt/trn_rl_repo")

import numpy as np
import ml_dtypes

B, C_IN, C_OUT, E, KK = 4, 32, 64, 200000, 5
NCORES = 8

CE = 2048  # edges per chunk
WIN = 1 << 15  # index window (int16 limit)
NQUEUES = 4
SCRATCH = 8192  # per-queue descriptor-ring carveout bytes


def _sizes():
    EH = E // 2  # edges per core
    R = E + 1  # table rows (row 0 = zeros)
    NW = -(-R // WIN)
    NCH = -(-EH // CE)
    EPAD = NCH * CE
    N2 = 5 * CE
    return EH, R, NW, NCH, EPAD, N2


_PROG_CACHE = {}
_PATCHED = [False]


def _patch_lane_assignment():
    """Make Tile's DMASW sem-lane rotation queue-aware: SWDGE queue q only
    ever uses lanes {2q, 2q+1}.  The ucode locks each DMA-completion sem to
    one descriptor ring (queue); Tile's stock round-robin mixes queues onto
    one lane, which faults.  Runtime-only patch of this process."""
    if _PATCHED[0]:
        return
    import concourse.tile_sem_assignment as tsa
    from concourse import mybir

    orig = tsa.TileClockTick._assign_tick

    def _assign_tick(self, inst):
        qnum = getattr(inst, "queue_num", None)
        if (
            qnum is not None
            and inst.engine == mybir.EngineType.Pool
            and isinstance(inst, tsa.DMAInst)
        ):
            par = getattr(self, "_swdge_q_parity", None)
            if par is None:
                par = {}
                self._swdge_q_parity = par
            p = par.get(qnum, 0)
            par[qnum] = p ^ 1
            self.next_sw_dma_idx = 2 * qnum + p
        return orig(self, inst)

    tsa.TileClockTick._assign_tick = _assign_tick
    _PATCHED[0] = True


def _build(caps, reps=1, passthrough=False):
    key = (tuple(caps), E, CE, reps, passthrough)
    if key in _PROG_CACHE:
        return _PROG_CACHE[key]
    import concourse.bass as bass
    import concourse.bacc as bacc
    import concourse.tile as tile
    from concourse import mybir

    _patch_lane_assignment()

    EH, R, NW, NCH, EPAD, N2 = _sizes()
    TS = int(sum(caps))
    TQ = TS // 128
    NA, NB = 2 * CE, 3 * CE  # g2 split: slots j in {0,1} / {2,3,4}
    QN = 512 if CE % 512 == 0 else CE
    NQ = CE // QN

    dt = mybir.dt
    AT = mybir.AluOpType
    nc = bacc.Bacc(
        "TRN2",
        target_bir_lowering=False,
        debug=False,
        num_swdge_queues=NQUEUES,
        dynamic_dma_scratch_size=SCRATCH,
    )
    table_d = nc.dram_tensor("table", [R, 128], dt.bfloat16, kind="ExternalInput")
    g1_d = nc.dram_tensor("g1idx", [NCH, 128, TS // 16], dt.int16, kind="ExternalInput")
    g2a_d = nc.dram_tensor("g2a", [NCH, 128, NA // 16], dt.int16, kind="ExternalInput")
    g2b_d = nc.dram_tensor("g2b", [NCH, 128, NB // 16], dt.int16, kind="ExternalInput")
    wts_d = nc.dram_tensor("wts", [C_IN, 5 * C_OUT], dt.bfloat16, kind="ExternalInput")
    bias_d = nc.dram_tensor("bias", [C_OUT, 1], dt.float32, kind="ExternalInput")
    out_d = nc.dram_tensor("out", [C_OUT, EPAD], dt.float32, kind="ExternalOutput")

    if passthrough:
        with tile.TileContext(nc) as tc:
            with tc.tile_pool(name="pt", bufs=1) as ptp:
                z = ptp.tile([C_OUT, CE], dt.float32)
                nc.vector.memset(z[:], 0.0)
                for ch in range(NCH):
                    nc.sync.dma_start(out_d[:, ch * CE : (ch + 1) * CE], z[:])
        nc.compile()
        _PROG_CACHE[key] = nc
        return nc

    offs = np.concatenate([[0], np.cumsum(caps)]).astype(int)
    qctr = [0]  # g1 rotates queues 0-2; transpose (XBAR) calls own queue 3

    def qn():
        q = qctr[0] % (NQUEUES - 1)
        qctr[0] += 1
        return q

    with tile.TileContext(nc) as tc:
        with (
            tc.tile_pool(name="const", bufs=1) as cp,
            tc.tile_pool(name="idxp", bufs=3) as ip,
            tc.tile_pool(name="fsp", bufs=2) as fsp,
            tc.tile_pool(name="flp", bufs=2) as flp,
            tc.tile_pool(name="ttp", bufs=2) as ttp,
            tc.tile_pool(name="obp", bufs=2) as obp,
            tc.tile_pool(name="psp", bufs=4, space="PSUM") as pp,
        ):
            wts = cp.tile([C_IN, 5 * C_OUT], dt.bfloat16)
            nc.sync.dma_start(wts[:], wts_d[:])
            bias = cp.tile([C_OUT, 1], dt.float32)
            nc.sync.dma_start(bias[:], bias_d[:])

            for _ in range(reps):
                state = {}

                def emit_g1(ch):
                    it1 = ip.tile([128, TS // 16], dt.int16, tag="it1")
                    nc.sync.dma_start(it1[:], g1_d[ch])
                    it2a = ip.tile([128, NA // 16], dt.int16, tag="it2a")
                    nc.sync.dma_start(it2a[:], g2a_d[ch])
                    it2b = ip.tile([128, NB // 16], dt.int16, tag="it2b")
                    nc.sync.dma_start(it2b[:], g2b_d[ch])
                    fs = fsp.tile([128, TQ, 128], dt.bfloat16, tag="fs")
                    for w in range(NW):
                        cw = int(caps[w])
                        lo, hi = w * WIN, min(R, (w + 1) * WIN)
                        nc.gpsimd.dma_gather(
                            fs[:, offs[w] // 128 : offs[w + 1] // 128, :],
                            table_d[lo:hi],
                            it1[:, offs[w] // 16 : offs[w + 1] // 16],
                            cw,
                            cw,
                            128,
                            single_packet=False,
                            queue_num=qn(),
                        )
                    state[ch] = (fs, it2a, it2b)

                def emit_rest(ch):
                    fs, it2a, it2b = state.pop(ch)
                    fla = flp.tile([128, 1, NA], dt.bfloat16, tag="fla")
                    flb = flp.tile([128, 1, NB], dt.bfloat16, tag="flb")
                    for fl, it2, nn in ((fla, it2a, NA), (flb, it2b, NB)):
                        nc.gpsimd.dma_gather(
                            fl[:],
                            fs[:],
                            it2[:],
                            nn,
                            nn,
                            128,
                            transpose=True,
                            sbuf_tokens_per_rank=128,
                            sbuf_free_dim_per_rank=256,
                            sbuf_free_dim_pad_per_rank=0,
                            sbuf_byte_offset=0,
                            single_packet=False,
                            queue_num=NQUEUES - 1,
                        )

                    def f(j):
                        src = fla if j < 2 else flb
                        jj = j if j < 2 else j - 2
                        return src[0:C_IN, 0, jj * CE : (jj + 1) * CE]

                    tt = ttp.tile([C_IN, 4 * CE], dt.bfloat16, tag="tt")
                    sc = ttp.tile([C_IN, 2 * CE], dt.bfloat16, tag="sc")
                    nc.vector.tensor_tensor(
                        out=tt[:, 0:CE], in0=f(1), in1=f(3), op=AT.add
                    )
                    nc.vector.tensor_tensor(
                        out=tt[:, CE : 2 * CE], in0=f(2), in1=f(4), op=AT.add
                    )
                    nc.vector.tensor_tensor(
                        out=sc[:, 0:CE], in0=f(1), in1=f(3), op=AT.subtract
                    )
                    nc.scalar.activation(
                        tt[:, 2 * CE : 3 * CE],
                        sc[:, 0:CE],
                        mybir.ActivationFunctionType.Abs,
                    )
                    nc.vector.tensor_tensor(
                        out=sc[:, CE : 2 * CE], in0=f(2), in1=f(4), op=AT.subtract
                    )
                    nc.scalar.activation(
                        tt[:, 3 * CE : 4 * CE],
                        sc[:, CE : 2 * CE],
                        mybir.ActivationFunctionType.Abs,
                    )

                    ob = obp.tile([C_OUT, CE], dt.float32, tag="ob")
                    for q in range(NQ):
                        ps = pp.tile([C_OUT, QN], dt.float32, tag="ps")
                        nc.tensor.matmul(
                            ps[:],
                            wts[:, 0:C_OUT],
                            fla[0:C_IN, 0, q * QN : (q + 1) * QN],
                            start=True,
                            stop=False,
                        )
                        for k in range(1, 5):
                            nc.tensor.matmul(
                                ps[:],
                                wts[:, k * C_OUT : (k + 1) * C_OUT],
                                tt[
                                    :,
                                    (k - 1) * CE + q * QN : (k - 1) * CE + (q + 1) * QN,
                                ],
                                start=False,
                                stop=(k == 4),
                            )
                        nc.vector.tensor_scalar(
                            out=ob[:, q * QN : (q + 1) * QN],
                            in0=ps[:],
                            scalar1=bias[:],
                            scalar2=None,
                            op0=AT.add,
                        )
                    nc.sync.dma_start(out_d[:, ch * CE : (ch + 1) * CE], ob[:])

                for ch in range(NCH + 1):
                    if ch < NCH:
                        emit_g1(ch)
                    if ch >= 1:
                        emit_rest(ch - 1)
    nc.compile()
    _PROG_CACHE[key] = nc
    return nc


def _wrap16(a):
    """(NCH, N) int16 -> (NCH, 128, N//16): index i -> [p, i//16] for all
    p % 16 == i % 16 (value wrapped in 16 partitions, replicated 8x)."""
    nch, n = a.shape
    b = a.reshape(nch, n // 16, 16).transpose(0, 2, 1)  # (NCH, 16, N//16)
    return np.ascontiguousarray(np.tile(b, (1, 8, 1)))


def _marshal_core(Gi_core, caps, offs):
    """Build g1 (window-bucketed gather lists) and g2 (un-permute) index
    arrays for one core from its (EH, 5) one-based token indices."""
    EH, R, NW, NCH, EPAD, N2 = _sizes()
    WSH = WIN.bit_length() - 1
    TS = int(sum(caps))
    gp = np.zeros((EPAD, KK), np.int64)
    gp[:EH] = Gi_core
    tok = gp.reshape(NCH, CE, KK).transpose(0, 2, 1).reshape(NCH, N2)
    w = tok >> WSH
    g1 = np.zeros((NCH, TS), np.int16)
    i2 = np.empty((NCH, N2), np.int16)
    for ch in range(NCH):
        tw, tt = w[ch], tok[ch]
        for wi in range(NW):
            m = tw == wi
            nn = int(m.sum())
            g1[ch, offs[wi] : offs[wi] + nn] = (tt[m] - (wi << WSH)).astype(np.int16)
            i2[ch, m] = (offs[wi] + np.arange(nn)).astype(np.int16)
    return _wrap16(g1), _wrap16(i2[:, : 2 * CE]), _wrap16(i2[:, 2 * CE :])


def _counts_core(Gi_core):
    EH, R, NW, NCH, EPAD, N2 = _sizes()
    WSH = WIN.bit_length() - 1
    gp = np.zeros((EPAD, KK), np.int64)
    gp[:EH] = Gi_core
    w = (gp.reshape(NCH, CE, KK).transpose(0, 2, 1).reshape(NCH, N2)) >> WSH
    return np.stack([np.bincount(w[ch], minlength=NW) for ch in range(NCH)])


def _marshal_weights(W, b):
    Wk = np.asarray(W)[:, :, 0, :]  # (C_OUT, C_IN, 5)
    wts = np.zeros((C_IN, 5 * C_OUT), np.float32)
    for k in range(KK):
        wts[:, k * C_OUT : (k + 1) * C_OUT] = Wk[:, :, k].T
    bias = np.asarray(b).reshape(C_OUT, 1).astype(np.float32)
    return wts.astype(ml_dtypes.bfloat16), bias


def _run(x, Gi, W, b, reps=1, passthrough=False):
    from concourse.bass_utils import run_bass_kernel_spmd

    EH, R, NW, NCH, EPAD, N2 = _sizes()
    x = np.asarray(x)
    Gi = np.asarray(Gi)

    gi1 = {}  # one-based token index per core
    for c in range(NCORES):
        bb, h = divmod(c, 2)
        gi1[c] = Gi[bb, h * EH : (h + 1) * EH].astype(np.int64) + 1

    # fixed per-window caps: max bucket size over every (core, chunk)
    cnts = np.stack([_counts_core(gi1[c]) for c in range(NCORES)])  # (8, NCH, NW)
    caps = (-(-cnts.max(axis=(0, 1)) // 128) * 128).astype(int)
    offs = np.concatenate([[0], np.cumsum(caps)]).astype(int)

    nc = _build(caps, reps, passthrough)
    wts, bias = _marshal_weights(W, b)

    tables = {}
    for bb in range(B):
        t = np.zeros((R, 128), ml_dtypes.bfloat16)
        t[1:, 0:C_IN] = x[bb].T.astype(ml_dtypes.bfloat16)
        tables[bb] = t

    in_maps = []
    for c in range(NCORES):
        bb, h = divmod(c, 2)
        g1, g2a, g2b = _marshal_core(gi1[c], caps, offs)
        in_maps.append(
            {
                "table": tables[bb],
                "g1idx": g1,
                "g2a": g2a,
                "g2b": g2b,
                "wts": wts,
                "bias": bias,
            }
        )
    res = run_bass_kernel_spmd(nc, in_maps, core_ids=list(range(NCORES)))
    out = np.empty((B, C_OUT, E, 1), np.float32)
    for c in range(NCORES):
        bb, h = divmod(c, 2)
        out[bb, :, h * EH : (h + 1) * EH, 0] = res.results[c]["out"][:, :EH]
    return out


def kernel(x, Gi, W, b):
    return _run(x, Gi, W, b)
